# revision 1
# baseline (speedup 1.0000x reference)
"""AAM-Softmax (ArcFace) logits kernel for Trainium2, 8 NeuronCores.

Math (per reference):
    cosine = l2norm(input) @ l2norm(weight).T            # [B, C]
    tgt    = cosine[i, label[i]]
    phi    = tgt*cos(m) - sqrt(1-tgt^2)*sin(m)
    out    = S * cosine, except out[i, label[i]] = S * where(tgt>0, phi, tgt)

Sharding: weight/cosine column-sharded over 8 cores (vocab parallel);
input + labels replicated.  Core k owns classes [k*CS, (k+1)*CS).

v5 pipeline (dense-queue design; output stored bf16, host upcasts —
rel tolerance 2e-2 dwarfs bf16 rounding, and it halves the dominant
HBM write traffic):
  - x ships twice: rows f32 (norms + margin dot) and pre-transposed
    x.T bf16 (host relayout) straight into the matmul operand.
  - per-sample S/||x|| folds into the PSUM->staging copies as a
    per-partition scalar (ACT scalar.mul / DVE tensor_scalar_mul);
    per-class 1/||w|| pre-folds into bf16 weights on gpsimd (DVE for
    group 0 to shorten the prologue chain).
  - uneven class-groups (small first and last) so the pipeline fills
    and drains fast; weight DMA staggered 3 groups ahead; group prep
    (squares -> ones-matmul norm^2 -> sqrt -> fast reciprocal -> fold)
    emitted inside the bi-loop two groups ahead.
  - margin target dot uses x.w = ((x+w)^2 - x^2 - w^2)/2 so the row
    sums ride the ACT square+accumulate path (no DVE reduce chain);
    scatter offsets are fully host-encoded (OOB sentinel off-shard)
    and the per-block scatters fire inside the last group's bi-loop.
"""

import sys

if "/opt/trn_rl_repo" not in sys.path:
    sys.path.insert(0, "/opt/trn_rl_repo")

from dataclasses import dataclass

import ml_dtypes
import numpy as np

S = 50.0
MARGIN = 0.5
COS_M = float(np.cos(MARGIN))
SIN_M = float(np.sin(MARGIN))
OOB = 2**30  # > any valid flat offset
GSIZES = (2, 5, 5, 5, 5, 3)  # c-tiles per group; sum == nct


@dataclass(frozen=True)
class Cfg:
    b: int = 1024
    d: int = 256
    c: int = 100000
    ncores: int = 8
    tc: int = 500

    @property
    def cs(self):
        return self.c // self.ncores

    @property
    def nb(self):
        return self.b // 128

    @property
    def nkt(self):
        return self.d // 128

    @property
    def nct(self):
        return self.cs // self.tc

    @property
    def gstarts(self):
        out = [0]
        for s in GSIZES:
            out.append(out[-1] + s)
        assert out[-1] == self.nct
        return out  # tile index starts, len ngr+1


def build(cfg: Cfg):
    import concourse.bass as bass
    import concourse.tile as tile
    from concourse import bacc, mybir

    f32 = mybir.dt.float32
    bf16 = mybir.dt.bfloat16
    i32 = mybir.dt.int32
    Op = mybir.AluOpType
    Act = mybir.ActivationFunctionType

    b, d, cs, tc = cfg.b, cfg.d, cfg.cs, cfg.tc
    nb, nkt = cfg.nb, cfg.nkt
    ngr = len(GSIZES)
    gst = cfg.gstarts

    nc = bacc.Bacc(
        "TRN2", target_bir_lowering=False, debug=False, num_devices=cfg.ncores
    )

    x_ext = nc.dram_tensor("x", [b, d], f32, kind="ExternalInput")
    xt_ext = nc.dram_tensor("xt", [nkt, 128, b], bf16, kind="ExternalInput")
    wt_ext = nc.dram_tensor("wt", [nkt, 128, cs], bf16, kind="ExternalInput")
    wsel_ext = nc.dram_tensor("wsel", [b, d], f32, kind="ExternalInput")
    labrel_ext = nc.dram_tensor("labrel", [128, nb], i32, kind="ExternalInput")
    # flat per-block outputs (indirect-DMA dynamic APs need offset 0);
    # group g of block bi lives at [gst[g]*tc*128 : gst[g+1]*tc*128)
    out_blocks = [
        nc.dram_tensor(f"out{bi}", [cs * 128], bf16, kind="ExternalOutput")
        for bi in range(nb)
    ]

    with tile.TileContext(nc) as tc_:
        with (
            tc_.tile_pool(name="const", bufs=1) as constp,
            tc_.tile_pool(name="persist", bufs=1) as persist,
            tc_.tile_pool(name="xin", bufs=nb) as xin,
            tc_.tile_pool(name="wsin", bufs=nb) as wsin,
            tc_.tile_pool(name="xsc", bufs=2) as xsc,
            tc_.tile_pool(name="wstream", bufs=8) as wstream,
            tc_.tile_pool(name="wt2p", bufs=4) as wt2p,
            tc_.tile_pool(name="wlogp", bufs=2) as wlogp,
            tc_.tile_pool(name="winvp", bufs=2) as winvp,
            tc_.tile_pool(name="wbfp", bufs=6) as wbfp,
            tc_.tile_pool(name="stage", bufs=6) as stage,
            tc_.tile_pool(name="pn", bufs=2, space="PSUM") as pn,
            tc_.tile_pool(name="po", bufs=6, space="PSUM") as po,
        ):
            ones_bf = constp.tile([128, 128], bf16)
            nc.vector.memset(ones_bf[:], 1.0)

            # persistent tensors
            xT = persist.tile([128, nkt * b], bf16)  # [d-half][k*b + i]
            offs_i = persist.tile([128, nb], i32)  # host-encoded offsets
            ss8 = persist.tile([128, nb], f32)
            wss8 = persist.tile([128, nb], f32)
            sum8 = persist.tile([128, nb], f32)
            rawdot8 = persist.tile([128, nb], f32)
            xnS8 = persist.tile([128, nb], f32)
            xinvS8 = persist.tile([128, nb], f32)
            xinv8 = persist.tile([128, nb], f32)
            wn8 = persist.tile([128, nb], f32)
            wsinv8 = persist.tile([128, nb], f32)
            newv8 = persist.tile([128, nb], bf16)

            # ---- weight stream state ----
            wt_f_g = {}  # g -> [k] tiles [128, gw] bf16
            wbf_g = {}  # g -> [k] folded bf16 tiles [128, gw]

            def gw_of(g):
                return GSIZES[g] * tc

            def wt_dma(g):
                gw = gw_of(g)
                c0 = gst[g] * tc
                ks = []
                for k in range(nkt):
                    wt_f = wstream.tile(
                        [128, gw], bf16, tag="wt_f", name="wt_f", bufs=8
                    )
                    nc.sync.dma_start(wt_f[:], wt_ext[k, :, c0 : c0 + gw])
                    ks.append(wt_f)
                wt_f_g[g] = ks

            def prep(g, fold_engine):
                gw = gw_of(g)
                # squares: k0 on DVE (bf16 2x), k1 on gpsimd to unload DVE
                w2s = []
                for k in range(nkt):
                    wt2 = wt2p.tile(
                        [128, gw], bf16, tag="wt2", name="wt2", bufs=4
                    )
                    sq_eng = nc.vector if k == 0 else fold_engine
                    sq_eng.tensor_tensor(
                        wt2[:], wt_f_g[g][k][:], wt_f_g[g][k][:], Op.mult
                    )
                    w2s.append(wt2)
                # column norm^2 broadcast over partitions (ones-matmul),
                # sqrt (ACT) per tile; reciprocal + fold in <=1250-col
                # chunks so no engine sees a multi-us monolith on the
                # prep chain
                wlog = wlogp.tile(
                    [128, gw], f32, tag="wlog", name="wlog", bufs=2
                )
                for ci in range(GSIZES[g]):
                    sl = slice(ci * tc, (ci + 1) * tc)
                    nps = pn.tile([128, tc], f32, tag="nps", name="nps")
                    for k in range(nkt):
                        nc.tensor.matmul(
                            nps[:],
                            lhsT=ones_bf[:],
                            rhs=w2s[k][:, sl],
                            start=(k == 0),
                            stop=(k == nkt - 1),
                        )
                    nc.scalar.activation(wlog[:, sl], nps[:], Act.Sqrt)
                winv = winvp.tile(
                    [128, gw], f32, tag="winv", name="winv", bufs=2
                )
                ck = 1250
                for off in range(0, gw, ck):
                    w = min(ck, gw - off)
                    nc.vector.reciprocal_approx_fast(
                        winv[:, off : off + w], wlog[:, off : off + w]
                    )
                ks = []
                for k in range(nkt):
                    wbf = wbfp.tile(
                        [128, gw], bf16, tag="wbf", name="wbf", bufs=6
                    )
                    for off in range(0, gw, ck):
                        w = min(ck, gw - off)
                        fold_engine.tensor_tensor(
                            wbf[:, off : off + w],
                            wt_f_g[g][k][:, off : off + w],
                            winv[:, off : off + w],
                            Op.mult,
                        )
                    ks.append(wbf)
                wbf_g[g] = ks

            # ---- prologue ----
            wt_dma(0)
            for k in range(nkt):
                nc.sync.dma_start(xT[:, k * b : (k + 1) * b], xt_ext[k])
            x_tiles = []
            for bi in range(nb):
                x_t = xin.tile([128, d], f32, tag="x_t", name="x_t", bufs=nb)
                nc.sync.dma_start(x_t[:], x_ext[bi * 128 : (bi + 1) * 128, :])
                x_tiles.append(x_t)
            nc.sync.dma_start(offs_i[:], labrel_ext[:])

            prep(0, nc.vector)

            wt_dma(1)
            wt_dma(2)
            ws_tiles = []
            for bi in range(nb):
                ws_t = wsin.tile(
                    [128, d], f32, tag="ws_t", name="ws_t", bufs=nb
                )
                nc.sync.dma_start(
                    ws_t[:], wsel_ext[bi * 128 : (bi + 1) * 128, :]
                )
                ws_tiles.append(ws_t)

            # x prep: row sumsq via ACT square+accumulate;
            # xinvS = S/||x|| feeds the copies as per-partition scale
            for bi in range(nb):
                sq = xsc.tile([128, d], f32, tag="sq", name="sq", bufs=2)
                nc.scalar.activation(
                    sq[:], x_tiles[bi][:], Act.Square,
                    accum_out=ss8[:, bi : bi + 1],
                )
            nc.scalar.activation(
                xnS8[:], ss8[:], Act.Sqrt, scale=1.0 / (S * S)
            )
            nc.vector.reciprocal(xinvS8[:], xnS8[:])
            nc.vector.tensor_scalar_mul(xinv8[:], xinvS8[:], 1.0 / S)

            prep(1, nc.gpsimd)

            # wsel row sumsq (ACT accumulate; late prologue, off the
            # critical path)
            for bi in range(nb):
                sq = xsc.tile([128, d], f32, tag="sq", name="sq", bufs=2)
                nc.scalar.activation(
                    sq[:], ws_tiles[bi][:], Act.Square,
                    accum_out=wss8[:, bi : bi + 1],
                )

            # ---- main loop ----
            for g in range(ngr):
                for bi in range(nb):
                    if bi == 1:
                        if g + 3 < ngr:
                            wt_dma(g + 3)
                        if g + 2 < ngr:
                            prep(g + 2, nc.gpsimd)
                        if g == 2:
                            # (x+w)^2 row sums: gpsimd add + ACT
                            # square-accumulate (keeps DVE out of it)
                            for bj in range(nb):
                                xw = xsc.tile(
                                    [128, d], f32, tag="xw", name="xw",
                                    bufs=2,
                                )
                                nc.gpsimd.tensor_tensor(
                                    xw[:], x_tiles[bj][:], ws_tiles[bj][:],
                                    Op.add,
                                )
                                sq = xsc.tile(
                                    [128, d], f32, tag="sq", name="sq",
                                    bufs=2,
                                )
                                nc.scalar.activation(
                                    sq[:], xw[:], Act.Square,
                                    accum_out=sum8[:, bj : bj + 1],
                                )
                        if g == 3:
                            # rawdot = ((x+w)^2 - x^2 - w^2) / 2
                            nc.vector.tensor_sub(rawdot8[:], sum8[:], ss8[:])
                            nc.vector.tensor_sub(
                                rawdot8[:], rawdot8[:], wss8[:]
                            )
                            nc.vector.tensor_scalar_mul(
                                rawdot8[:], rawdot8[:], 0.5
                            )
                            nc.scalar.activation(wn8[:], wss8[:], Act.Sqrt)
                            nc.vector.reciprocal(wsinv8[:], wn8[:])
                    if g == ngr - 2 and bi == 0:
                        # margin math (tiny [128, nb] chain) -- early so
                        # the last group's scatters pipeline with drain
                        tgt8 = persist.tile([128, nb], f32)
                        nc.vector.tensor_mul(tgt8[:], rawdot8[:], xinv8[:])
                        nc.vector.tensor_mul(tgt8[:], tgt8[:], wsinv8[:])
                        tsq = persist.tile([128, nb], f32)
                        nc.vector.tensor_mul(tsq[:], tgt8[:], tgt8[:])
                        om = persist.tile([128, nb], f32)
                        nc.vector.tensor_scalar(
                            om[:], tsq[:], -1.0, 1.0, Op.mult, Op.add
                        )
                        nc.vector.tensor_scalar_max(om[:], om[:], 0.0)
                        sine8 = persist.tile([128, nb], f32)
                        nc.scalar.activation(sine8[:], om[:], Act.Sqrt)
                        phi8 = persist.tile([128, nb], f32)
                        nc.vector.tensor_scalar_mul(phi8[:], tgt8[:], COS_M)
                        ssin8 = persist.tile([128, nb], f32)
                        nc.vector.tensor_scalar_mul(ssin8[:], sine8[:], SIN_M)
                        nc.vector.tensor_sub(phi8[:], phi8[:], ssin8[:])
                        mask8 = persist.tile([128, nb], mybir.dt.uint8)
                        nc.vector.tensor_scalar(
                            mask8[:], tgt8[:], 0.0, None, Op.is_gt
                        )
                        selv8 = persist.tile([128, nb], f32)
                        nc.vector.select(selv8[:], mask8[:], phi8[:], tgt8[:])
                        nc.vector.tensor_scalar_mul(newv8[:], selv8[:], S)

                    gw = gw_of(g)
                    sz = GSIZES[g]
                    ops_g = [
                        po.tile([128, tc], f32, tag="ops", name="ops")
                        for _ in range(sz)
                    ]
                    for k in range(nkt):
                        col = k * b + bi * 128
                        lhs = xT[:, col : col + 128]
                        for ci in range(sz):
                            nc.tensor.matmul(
                                ops_g[ci][:],
                                lhsT=lhs,
                                rhs=wbf_g[g][k][:, ci * tc : (ci + 1) * tc],
                                start=(k == 0),
                                stop=(k == nkt - 1),
                            )
                    stw = stage.tile(
                        [128, gw], bf16, tag="stw", name="stw", bufs=6
                    )
                    xsc_ap = xinvS8[:, bi : bi + 1]
                    for ci in range(sz):
                        dst = stw[:, ci * tc : (ci + 1) * tc]
                        # ~5/9 of the PSUM drain on ACT, rest on DVE
                        if (ci + bi) % 9 < 5:
                            nc.scalar.mul(dst, ops_g[ci][:], xsc_ap)
                        else:
                            nc.vector.tensor_scalar_mul(
                                dst, ops_g[ci][:], xsc_ap
                            )
                    dst = out_blocks[bi][
                        gst[g] * tc * 128 : gst[g + 1] * tc * 128
                    ].rearrange("(r c) -> r c", r=128)
                    nc.sync.dma_start(dst, stw[:])
                    if g == ngr - 1:
                        # per-block margin scatter: depends only on this
                        # block's bulk DMAs, pipelines with the drain
                        out_flat = out_blocks[bi][:].rearrange(
                            "(c one) -> c one", one=1
                        )
                        nc.gpsimd.indirect_dma_start(
                            out=out_flat,
                            out_offset=bass.IndirectOffsetOnAxis(
                                ap=offs_i[:, bi : bi + 1], axis=0
                            ),
                            in_=newv8[:, bi : bi + 1],
                            in_offset=None,
                            bounds_check=cs * 128 - 1,
                            oob_is_err=False,
                        )

    nc.compile()
    return nc


def host_prep(cfg: Cfg, input, label, weight):
    x = np.ascontiguousarray(np.asarray(input, dtype=np.float32))
    xt = np.ascontiguousarray(
        x.T.reshape(cfg.nkt, 128, cfg.b).astype(ml_dtypes.bfloat16)
    )
    w = np.asarray(weight, dtype=np.float32)
    lab = np.asarray(label).astype(np.int64)
    wsel = np.ascontiguousarray(w[lab])
    wt_all = np.ascontiguousarray(w.T)  # [D, C], relayout only
    gst = np.array(cfg.gstarts) * cfg.tc  # class offsets per group
    gsz = np.array(GSIZES) * cfg.tc
    p_of = np.arange(cfg.b) % 128
    in_maps = []
    for core in range(cfg.ncores):
        sl = slice(core * cfg.cs, (core + 1) * cfg.cs)
        wt = (
            np.ascontiguousarray(wt_all[:, sl])
            .reshape(cfg.nkt, 128, cfg.cs)
            .astype(ml_dtypes.bfloat16)
        )
        rel = lab - core * cfg.cs
        inb = (rel >= 0) & (rel < cfg.cs)
        relc = np.where(inb, rel, 0)
        g_of = np.searchsorted(gst, relc, side="right") - 1
        col_of = relc - gst[g_of]
        off = gst[g_of] * 128 + p_of * gsz[g_of] + col_of
        rel2 = np.where(inb, off, OOB).astype(np.int32)
        labrel = np.ascontiguousarray(rel2.reshape(cfg.nb, 128).T)
        in_maps.append(
            {"x": x, "xt": xt, "wt": wt, "wsel": wsel, "labrel": labrel}
        )
    return in_maps


def run(cfg: Cfg, nc, in_maps, **kw):
    from concourse.bass_utils import run_bass_kernel_spmd

    try:
        res = run_bass_kernel_spmd(
            nc, in_maps, core_ids=list(range(cfg.ncores)), **kw
        )
    except Exception:
        # rare transient device faults have been observed; retry once
        res = run_bass_kernel_spmd(
            nc, in_maps, core_ids=list(range(cfg.ncores)), **kw
        )
    out = np.empty((cfg.b, cfg.c), dtype=np.float32)
    gst = [s * cfg.tc for s in cfg.gstarts]
    for c in range(cfg.ncores):
        for bi in range(cfg.nb):
            flat = res.results[c][f"out{bi}"]  # [cs*128] bf16
            rows = slice(bi * 128, (bi + 1) * 128)
            for g in range(len(GSIZES)):
                gw = GSIZES[g] * cfg.tc
                seg = flat[gst[g] * 128 : gst[g + 1] * 128].reshape(128, gw)
                out[rows, c * cfg.cs + gst[g] : c * cfg.cs + gst[g + 1]] = (
                    seg.astype(np.float32)
                )
    return out, res


_cache = {}


def kernel(input, label, weight):
    cfg = Cfg()
    if cfg not in _cache:
        _cache[cfg] = build(cfg)
    in_maps = host_prep(cfg, input, label, weight)
    out, _ = run(cfg, _cache[cfg], in_maps)
    return out



# revision 4
# speedup vs baseline: 1.3696x; 1.3696x over previous
"""AAM-Softmax (ArcFace) logits kernel for Trainium2, 8 NeuronCores.

Math (per reference):
    cosine = l2norm(input) @ l2norm(weight).T            # [B, C]
    tgt    = cosine[i, label[i]]
    phi    = tgt*cos(m) - sqrt(1-tgt^2)*sin(m)
    out    = S * cosine, except out[i, label[i]] = S * where(tgt>0, phi, tgt)

Sharding: weight/cosine column-sharded over 8 cores (vocab parallel);
input + labels replicated.  Core k owns classes [k*CS, (k+1)*CS).

v6 design (memory-roofline focused; out stored bf16, host upcasts —
rel tolerance 2e-2 dwarfs bf16 rounding and it halves the dominant
HBM write traffic):
  - L2 normalization is folded into the host-side layout prep: device
    receives xt = (S * x/||x||).T bf16 and wt = (w/||w||).T bf16, so
    the device does a single dense bf16 GEMM + PSUM->SBUF cast + store.
    No on-device norm matmuls / sqrt / reciprocal / fold chains.
  - weights are fully SBUF-resident (50 KB/partition): streamed in via
    6 large DMAs once, then the PE runs one dense 400-matmul stream
    (g outer, bi inner) with all 8 PSUM banks in flight.
  - margin path: host ships xpw = bf16(x/||x|| + (w/||w||)[label]);
    since both addends are unit-norm, tgt = (rowsumsq(xpw) - 2)/2 via
    one ACT square+accumulate pass per block.  phi/select chain is a
    tiny [128, nb] vector job; host-encoded offsets (OOB sentinel
    off-shard) drive one per-block indirect scatter at the end.
"""

import sys

if "/opt/trn_rl_repo" not in sys.path:
    sys.path.insert(0, "/opt/trn_rl_repo")

from dataclasses import dataclass

import ml_dtypes
import numpy as np

S = 50.0
MARGIN = 0.5
COS_M = float(np.cos(MARGIN))
SIN_M = float(np.sin(MARGIN))
OOB = 2**30  # > any valid flat offset
GSIZES = (1, 8, 8, 8)  # c-tiles per group; sum == nct


@dataclass(frozen=True)
class Cfg:
    b: int = 1024
    d: int = 256
    c: int = 100000
    ncores: int = 8
    tc: int = 500

    @property
    def cs(self):
        return self.c // self.ncores

    @property
    def nb(self):
        return self.b // 128

    @property
    def nkt(self):
        return self.d // 128

    @property
    def nct(self):
        return self.cs // self.tc

    @property
    def gstarts(self):
        out = [0]
        for s in GSIZES:
            out.append(out[-1] + s)
        assert out[-1] == self.nct
        return out  # tile index starts, len ngr+1


def build(cfg: Cfg):
    import concourse.bass as bass
    import concourse.tile as tile
    from concourse import bacc, mybir

    f32 = mybir.dt.float32
    bf16 = mybir.dt.bfloat16
    i32 = mybir.dt.int32
    Op = mybir.AluOpType
    Act = mybir.ActivationFunctionType

    b, d, cs, tc = cfg.b, cfg.d, cfg.cs, cfg.tc
    nb, nkt = cfg.nb, cfg.nkt
    ngr = len(GSIZES)
    gst = cfg.gstarts

    nc = bacc.Bacc(
        "TRN2", target_bir_lowering=False, debug=False, num_devices=cfg.ncores
    )

    xt_ext = nc.dram_tensor("xt", [nkt, 128, b], bf16, kind="ExternalInput")
    wt_ext = nc.dram_tensor("wt", [nkt, 128, cs], bf16, kind="ExternalInput")
    xpw_ext = nc.dram_tensor("xpw", [128, nb * d], bf16, kind="ExternalInput")
    labrel_ext = nc.dram_tensor("labrel", [128, nb], i32, kind="ExternalInput")
    # flat per-block outputs (indirect-DMA dynamic APs need offset 0);
    # group g of block bi lives at [gst[g]*tc*128 : gst[g+1]*tc*128)
    out_blocks = [
        nc.dram_tensor(f"out{bi}", [cs * 128], bf16, kind="ExternalOutput")
        for bi in range(nb)
    ]

    with tile.TileContext(nc) as tc_:
        with (
            tc_.tile_pool(name="persist", bufs=1) as persist,
            tc_.tile_pool(name="sqp", bufs=2) as sqp,
            tc_.tile_pool(name="stage", bufs=4) as stage,
            tc_.tile_pool(name="po", bufs=8, space="PSUM") as po,
        ):
            # persistent tensors
            xt_k = [
                persist.tile([128, b], bf16, name=f"xt{k}")
                for k in range(nkt)
            ]
            xpw_t = persist.tile([128, nb * d], bf16)
            offs_i = persist.tile([128, nb], i32)  # host-encoded offsets
            sum8 = persist.tile([128, nb], f32)
            newv8 = persist.tile([128, nb], bf16)

            def gwc(g):
                return GSIZES[g] * tc

            # weights fully resident: one tile per (group, k)
            wsb = [
                [
                    persist.tile([128, gwc(g)], bf16, name=f"w{g}_{k}")
                    for k in range(nkt)
                ]
                for g in range(ngr)
            ]

            # ---- prologue DMAs ----
            nc.sync.dma_start(offs_i[:], labrel_ext[:])
            for k in range(nkt):
                nc.sync.dma_start(xt_k[k][:], xt_ext[k])
            for k in range(nkt):
                c0 = gst[0] * tc
                nc.sync.dma_start(
                    wsb[0][k][:], wt_ext[k, :, c0 : c0 + gwc(0)]
                )
            nc.sync.dma_start(xpw_t[:], xpw_ext[:])
            for g in range(1, ngr):
                c0 = gst[g] * tc
                for k in range(nkt):
                    nc.sync.dma_start(
                        wsb[g][k][:], wt_ext[k, :, c0 : c0 + gwc(g)]
                    )

            # ---- main loop: g outer (weights stream once), bi inner ----
            for g in range(ngr):
                if g == 1:
                    # margin math: tgt = (rowsumsq(xpw) - 2) / 2 since both
                    # addends are unit-norm.  Tiny [128, nb] chain, emitted
                    # early so the final scatters have their values ready.
                    for bi in range(nb):
                        sq = sqp.tile(
                            [128, d], f32, tag="sq", name="sq", bufs=2
                        )
                        nc.scalar.activation(
                            sq[:],
                            xpw_t[:, bi * d : (bi + 1) * d],
                            Act.Square,
                            accum_out=sum8[:, bi : bi + 1],
                        )
                    tgt8 = persist.tile([128, nb], f32)
                    nc.vector.tensor_scalar(
                        tgt8[:], sum8[:], -2.0, 0.5, Op.add, Op.mult
                    )
                    tsq = persist.tile([128, nb], f32)
                    nc.vector.tensor_mul(tsq[:], tgt8[:], tgt8[:])
                    om = persist.tile([128, nb], f32)
                    nc.vector.tensor_scalar(
                        om[:], tsq[:], -1.0, 1.0, Op.mult, Op.add
                    )
                    nc.vector.tensor_scalar_max(om[:], om[:], 0.0)
                    sine8 = persist.tile([128, nb], f32)
                    nc.scalar.activation(sine8[:], om[:], Act.Sqrt)
                    phi8 = persist.tile([128, nb], f32)
                    nc.vector.tensor_scalar_mul(phi8[:], tgt8[:], COS_M)
                    ssin8 = persist.tile([128, nb], f32)
                    nc.vector.tensor_scalar_mul(ssin8[:], sine8[:], SIN_M)
                    nc.vector.tensor_sub(phi8[:], phi8[:], ssin8[:])
                    mask8 = persist.tile([128, nb], mybir.dt.uint8)
                    nc.vector.tensor_scalar(
                        mask8[:], tgt8[:], 0.0, None, Op.is_gt
                    )
                    selv8 = persist.tile([128, nb], f32)
                    nc.vector.select(selv8[:], mask8[:], phi8[:], tgt8[:])
                    nc.vector.tensor_scalar_mul(newv8[:], selv8[:], S)

                sz = GSIZES[g]
                for bi in range(nb):
                    ops_g = [
                        po.tile([128, tc], f32, tag="ops", name="ops")
                        for _ in range(sz)
                    ]
                    for k in range(nkt):
                        lhs = xt_k[k][:, bi * 128 : (bi + 1) * 128]
                        for ci in range(sz):
                            nc.tensor.matmul(
                                ops_g[ci][:],
                                lhsT=lhs,
                                rhs=wsb[g][k][:, ci * tc : (ci + 1) * tc],
                                start=(k == 0),
                                stop=(k == nkt - 1),
                            )
                    stw = stage.tile(
                        [128, gwc(g)], bf16, tag="stw", name="stw", bufs=4
                    )
                    for ci in range(sz):
                        dst = stw[:, ci * tc : (ci + 1) * tc]
                        # ~3/8 of the PSUM drain on ACT, rest on DVE
                        if (ci + bi) % 8 < 3:
                            nc.scalar.copy(dst, ops_g[ci][:])
                        else:
                            nc.vector.tensor_scalar_mul(
                                dst, ops_g[ci][:], 1.0
                            )
                    dst = out_blocks[bi][
                        gst[g] * tc * 128 : gst[g + 1] * tc * 128
                    ].rearrange("(r c) -> r c", r=128)
                    nc.sync.dma_start(dst, stw[:])
                    if g == ngr - 1:
                        # per-block margin scatter: depends only on this
                        # block's bulk DMAs, pipelines with the drain
                        out_flat = out_blocks[bi][:].rearrange(
                            "(c one) -> c one", one=1
                        )
                        nc.gpsimd.indirect_dma_start(
                            out=out_flat,
                            out_offset=bass.IndirectOffsetOnAxis(
                                ap=offs_i[:, bi : bi + 1], axis=0
                            ),
                            in_=newv8[:, bi : bi + 1],
                            in_offset=None,
                            bounds_check=cs * 128 - 1,
                            oob_is_err=False,
                        )

    nc.compile()
    return nc


def host_prep(cfg: Cfg, input, label, weight):
    x = np.asarray(input, dtype=np.float32)
    w = np.asarray(weight, dtype=np.float32)
    lab = np.asarray(label).astype(np.int64)

    xn = x / np.maximum(
        np.sqrt(np.sum(x.astype(np.float64) ** 2, axis=1, keepdims=True)),
        1e-12,
    ).astype(np.float32)
    wn = w / np.maximum(
        np.sqrt(np.sum(w.astype(np.float64) ** 2, axis=1, keepdims=True)),
        1e-12,
    ).astype(np.float32)

    xt = np.ascontiguousarray(
        (S * xn).T.reshape(cfg.nkt, 128, cfg.b).astype(ml_dtypes.bfloat16)
    )
    xpw = (xn + wn[lab]).astype(ml_dtypes.bfloat16)  # [b, d]
    xpw_t = np.ascontiguousarray(
        xpw.reshape(cfg.nb, 128, cfg.d)
        .transpose(1, 0, 2)
        .reshape(128, cfg.nb * cfg.d)
    )

    wt_all = wn.T  # [D, C] view, relayout below
    gst = np.array(cfg.gstarts) * cfg.tc  # class offsets per group
    gsz = np.array(GSIZES) * cfg.tc
    p_of = np.arange(cfg.b) % 128
    in_maps = []
    for core in range(cfg.ncores):
        sl = slice(core * cfg.cs, (core + 1) * cfg.cs)
        wt = (
            np.ascontiguousarray(wt_all[:, sl])
            .reshape(cfg.nkt, 128, cfg.cs)
            .astype(ml_dtypes.bfloat16)
        )
        rel = lab - core * cfg.cs
        inb = (rel >= 0) & (rel < cfg.cs)
        relc = np.where(inb, rel, 0)
        g_of = np.searchsorted(gst, relc, side="right") - 1
        col_of = relc - gst[g_of]
        off = gst[g_of] * 128 + p_of * gsz[g_of] + col_of
        rel2 = np.where(inb, off, OOB).astype(np.int32)
        labrel = np.ascontiguousarray(rel2.reshape(cfg.nb, 128).T)
        in_maps.append(
            {"xt": xt, "wt": wt, "xpw": xpw_t, "labrel": labrel}
        )
    return in_maps


def run(cfg: Cfg, nc, in_maps, **kw):
    from concourse.bass_utils import run_bass_kernel_spmd

    try:
        res = run_bass_kernel_spmd(
            nc, in_maps, core_ids=list(range(cfg.ncores)), **kw
        )
    except Exception:
        # rare transient device faults have been observed; retry once
        res = run_bass_kernel_spmd(
            nc, in_maps, core_ids=list(range(cfg.ncores)), **kw
        )
    out = np.empty((cfg.b, cfg.c), dtype=np.float32)
    gst = [s * cfg.tc for s in cfg.gstarts]
    for c in range(cfg.ncores):
        for bi in range(cfg.nb):
            flat = res.results[c][f"out{bi}"]  # [cs*128] bf16
            rows = slice(bi * 128, (bi + 1) * 128)
            for g in range(len(GSIZES)):
                gw = GSIZES[g] * cfg.tc
                seg = flat[gst[g] * 128 : gst[g + 1] * 128].reshape(128, gw)
                out[rows, c * cfg.cs + gst[g] : c * cfg.cs + gst[g + 1]] = (
                    seg.astype(np.float32)
                )
    return out, res


_cache = {}


def kernel(input, label, weight):
    cfg = Cfg()
    if cfg not in _cache:
        _cache[cfg] = build(cfg)
    in_maps = host_prep(cfg, input, label, weight)
    out, _ = run(cfg, _cache[cfg], in_maps)
    return out
